# revision 2
# baseline (speedup 1.0000x reference)
"""Trainium2 Bass kernel v2 for the DenoisingModule (non-local attention).

Math (see reference):
    theta = Wt @ x / 16            [B, 128, HW]   (1/16 folded host-side)
    phi   = Wp @ x                 [B, 128, HW]
    f     = theta^T @ phi          [B, HW, HW]
    fh    = softmax(f, axis=0)     (over the BATCH axis - PyTorch legacy dim=0)
    den   = fh @ x^T               [B, C, HW]
    out   = den + (Wc @ den + bc)  = (I + Wc) @ den + bc

Sharding: n-axis (rows of f) across 8 cores; softmax over b is local.
No collectives; host slices inputs / concatenates outputs.

v2 changes vs v1 (1.05 ms):
  * bf16 datapath for x / theta / phi / fexp: DVE gets the 2x packed mode
    for the softmax adds+muls (the v1 bottleneck), DMA bytes halve.
  * Batched DMA: one strided DMA per (m-chunk, layout) instead of 12
    (426 -> ~30 DMA instructions; each real DMA has ~0.6-2us fixed cost).
  * m-chunks processed in per-core ROTATED order (host pre-rotates the
    arrays) so chunk 0 is the core's own n-range: theta reuses it and the
    xs input disappears.
  * S kept bf16 except the final tree add (f32) -> R = exp(-ln S) on ACT
    (Exp/Ln/Identity share one act table set -> no table reloads).
  * Per-b tile pools so the chunk pipeline frees slots b-by-b.
"""

import sys

import numpy as np

B = 8
C = 256
D = C // 2  # 128
HW = 4096
NCORES = 8
NLOC = HW // NCORES  # 512 n-columns per core
MC = 512  # m-chunk size
NCHUNK = HW // MC  # 8
P = 128
NSPL = MC // P  # 4 m-subtiles per chunk

TRACE = False
LAST = {}

_prog = None


def _ensure_path():
    try:
        import concourse  # noqa: F401
    except ImportError:
        for p in ("/opt/trn_rl_repo", "/root/.axon_site/_ro/trn_rl_repo"):
            if p not in sys.path:
                sys.path.insert(0, p)
        import concourse  # noqa: F401


def _build():
    from contextlib import ExitStack

    import concourse.bass as bass
    import concourse.tile as tile
    from concourse import mybir

    f32 = mybir.dt.float32
    f32r = mybir.dt.float32r
    bf16 = mybir.dt.bfloat16
    AF = mybir.ActivationFunctionType

    nc = bass.Bass(trn_type="TRN2", target_bir_lowering=False, debug=False)

    xn_h = nc.dram_tensor("xn", [B, C, HW], bf16, kind="ExternalInput")
    xt_h = nc.dram_tensor("xt", [B, HW, C], bf16, kind="ExternalInput")
    wthT_h = nc.dram_tensor("wthT", [C, D], bf16, kind="ExternalInput")
    wphT_h = nc.dram_tensor("wphT", [C, D], bf16, kind="ExternalInput")
    wcT_h = nc.dram_tensor("wcT", [C, C], f32r, kind="ExternalInput")
    bth_h = nc.dram_tensor("bth", [D, 1], f32, kind="ExternalInput")
    bph_h = nc.dram_tensor("bph", [D, 1], f32, kind="ExternalInput")
    bc_h = nc.dram_tensor("bc", [C, 1], f32, kind="ExternalInput")
    out_h = nc.dram_tensor("out", [B, C, NLOC], f32, kind="ExternalOutput")

    with tile.TileContext(nc) as tc:
        with ExitStack() as ctx:
            consts = ctx.enter_context(tc.tile_pool(name="consts", bufs=1))
            theta_p = ctx.enter_context(tc.tile_pool(name="theta", bufs=1))
            xn_p = ctx.enter_context(tc.tile_pool(name="xnp", bufs=16))
            xt_p = ctx.enter_context(tc.tile_pool(name="xtp", bufs=8))
            phi_p = ctx.enter_context(tc.tile_pool(name="phip", bufs=10))
            fexp_p = ctx.enter_context(tc.tile_pool(name="fexpp", bufs=9))
            smx_p = ctx.enter_context(tc.tile_pool(name="smxp", bufs=2))
            den_p = ctx.enter_context(tc.tile_pool(name="denp", bufs=1))
            out_p = ctx.enter_context(tc.tile_pool(name="outp", bufs=2))
            psA = ctx.enter_context(tc.tile_pool(name="psA", bufs=4, space="PSUM"))
            psD = ctx.enter_context(tc.tile_pool(name="psD", bufs=2, space="PSUM"))

            # ---- constants ----
            wth_sb = []
            wph_sb = []
            wc_sb = []
            for ck in range(2):
                t = consts.tile([P, D], bf16, name=f"wth{ck}", tag=f"wth{ck}")
                nc.sync.dma_start(out=t, in_=wthT_h.ap()[ck * P:(ck + 1) * P, :])
                wth_sb.append(t)
                t = consts.tile([P, D], bf16, name=f"wph{ck}", tag=f"wph{ck}")
                nc.sync.dma_start(out=t, in_=wphT_h.ap()[ck * P:(ck + 1) * P, :])
                wph_sb.append(t)
                t = consts.tile([P, C], f32r, name=f"wc{ck}", tag=f"wc{ck}")
                nc.sync.dma_start(out=t, in_=wcT_h.ap()[ck * P:(ck + 1) * P, :])
                wc_sb.append(t)
            bth_sb = consts.tile([D, 1], f32, name="bth", tag="bth")
            nc.sync.dma_start(out=bth_sb, in_=bth_h.ap()[:, :])
            bph_sb = consts.tile([D, 1], f32, name="bph", tag="bph")
            nc.sync.dma_start(out=bph_sb, in_=bph_h.ap()[:, :])
            bc_sb = []
            for dk in range(2):
                t = consts.tile([P, 1], f32, name=f"bc{dk}", tag=f"bc{dk}")
                nc.sync.dma_start(out=t, in_=bc_h.ap()[dk * P:(dk + 1) * P, :])
                bc_sb.append(t)

            # chunk input tiles: few big strided DMAs (per-b for fast rampup)
            def load_xn(mc):
                ts = []
                for b in range(B):
                    t = xn_p.tile([P, 2, MC], bf16, name=f"xn{mc}_{b}", tag="xn")
                    src = xn_h.ap()[b, :, mc * MC:(mc + 1) * MC]
                    src = src.rearrange("(ck p) m -> p ck m", ck=2)
                    nc.sync.dma_start(out=t, in_=src)
                    ts.append(t)
                return ts

            def load_xt(mc):
                ts = []
                for s in range(NSPL):
                    t = xt_p.tile([P, B, C], bf16, name=f"xt{mc}_{s}", tag="xt")
                    m0 = mc * MC + s * P
                    src = xt_h.ap()[:, m0:m0 + P, :]
                    src = src.rearrange("b p c -> p b c")
                    nc.sync.dma_start(out=t, in_=src)
                    ts.append(t)
                return ts

            xn_t0 = load_xn(0)
            xt_t0 = load_xt(0)

            # ---- theta from (rotated) chunk 0 == this core's n-range ----
            theta_sb = []
            for b in range(B):
                ps = psA.tile([P, NLOC], f32, name=f"psth{b}", tag="psA")
                for ck in range(2):
                    nc.tensor.matmul(
                        ps, wth_sb[ck], xn_t0[b][:, ck, :],
                        start=(ck == 0), stop=(ck == 1),
                    )
                th = theta_p.tile([D, NLOC], bf16, name=f"theta{b}",
                                  tag=f"theta{b}")
                nc.scalar.activation(th, ps, AF.Identity, bias=bth_sb)
                theta_sb.append(th)

            # ---- main loop over m-chunks (already rotated host-side) ----
            # den is LAGGED one chunk: den[k-1][b] matmuls are interleaved
            # with f~[k][b] so ACT's exps of chunk k overlap PE's den work
            # of chunk k-1 (and vice versa) instead of serializing.
            den_sb = [None] * B

            def den_group(mck, b, fh, xts):
                psd = psD.tile([P, 2 * NLOC], f32, name=f"psd{mck}_{b}",
                               tag="psD")
                for ct in range(2):
                    for s in range(NSPL):
                        nc.tensor.matmul(
                            psd[:, ct * NLOC:(ct + 1) * NLOC],
                            xts[s][:, b, ct * P:(ct + 1) * P],
                            fh[:, s, :],
                            start=(s == 0), stop=(s == NSPL - 1),
                        )
                if mck == 0:
                    dn = den_p.tile([P, 2 * NLOC], f32r, name=f"den{b}",
                                    tag=f"den{b}")
                    nc.vector.tensor_copy(dn, psd)
                    den_sb[b] = dn
                else:
                    nc.vector.tensor_add(den_sb[b], den_sb[b], psd)

            fh_prev = None
            xt_prev = None
            for mc in range(NCHUNK):
                xn_t = xn_t0 if mc == 0 else load_xn(mc)
                xt_t = xt_t0 if mc == 0 else load_xt(mc)

                # phi for this chunk: [d=128, m=512] per batch, bf16
                phi_sb = []
                for b in range(B):
                    ps = psA.tile([P, MC], f32, name=f"psph{mc}_{b}", tag="psA")
                    for ck in range(2):
                        nc.tensor.matmul(
                            ps, wph_sb[ck], xn_t[b][:, ck, :],
                            start=(ck == 0), stop=(ck == 1),
                        )
                    ph = phi_p.tile([D, MC], bf16, name=f"phi{mc}_{b}", tag="phi")
                    nc.scalar.activation(ph, ps, AF.Identity, bias=bph_sb)
                    phi_sb.append(ph)

                # fexp[k][b] = exp(phi_s^T theta); den[k-1][b] interleaved
                fexp = []
                t_l = []
                for b in range(B):
                    fe = fexp_p.tile([P, NSPL, NLOC], bf16, name=f"fexp{mc}_{b}",
                                     tag="fexp")
                    for s in range(NSPL):
                        ps = psA.tile([P, NLOC], f32, name=f"psf{mc}_{b}_{s}",
                                      tag="psA")
                        nc.tensor.matmul(
                            ps, phi_sb[b][:, s * P:(s + 1) * P], theta_sb[b],
                            start=True, stop=True,
                        )
                        nc.scalar.activation(fe[:, s, :], ps, AF.Exp)
                    fexp.append(fe)
                    if fh_prev is not None:
                        den_group(mc - 1, b, fh_prev[b], xt_prev)
                    if b % 2 == 1:
                        t = smx_p.tile([P, NSPL, NLOC], bf16,
                                       name=f"st{mc}_{b // 2}",
                                       tag=f"st{b // 2}", bufs=1)
                        nc.vector.tensor_add(t, fexp[b - 1], fexp[b])
                        t_l.append(t)

                # softmax over batch, sliced by s so the DVE->ACT->DVE chain
                # pipelines instead of serializing at the chunk boundary:
                # S = sum_b fexp[b] (bf16 tree, f32 top), R = exp(-ln S) bf16,
                # fh = fexp * R in place (bf16 2x).
                S = smx_p.tile([P, NSPL, NLOC], f32, name=f"S{mc}", tag="S",
                               bufs=1)
                lnS = smx_p.tile([P, NSPL, NLOC], f32, name=f"lnS{mc}",
                                 tag="lnS", bufs=1)
                R = smx_p.tile([P, NSPL, NLOC], bf16, name=f"R{mc}", tag="R")
                for s in range(NSPL):
                    nc.vector.tensor_add(t_l[0][:, s, :], t_l[0][:, s, :],
                                         t_l[1][:, s, :])
                    nc.vector.tensor_add(t_l[2][:, s, :], t_l[2][:, s, :],
                                         t_l[3][:, s, :])
                    nc.vector.tensor_add(S[:, s, :], t_l[0][:, s, :],
                                         t_l[2][:, s, :])
                    nc.scalar.activation(lnS[:, s, :], S[:, s, :], AF.Ln)
                    nc.scalar.activation(R[:, s, :], lnS[:, s, :], AF.Exp,
                                         scale=-1.0)
                    for b in range(B):
                        nc.vector.tensor_mul(fexp[b][:, s, :], fexp[b][:, s, :],
                                             R[:, s, :])

                fh_prev = fexp
                xt_prev = xt_t

            # tail: den for the last chunk, out stage interleaved per b
            for b in range(B):
                den_group(NCHUNK - 1, b, fh_prev[b], xt_prev)
                ot = out_p.tile([P, 2, NLOC], f32, name=f"out{b}", tag="out")
                for dk in range(2):
                    ps = psA.tile([P, NLOC], f32, name=f"pso{b}_{dk}", tag="psA")
                    for ct in range(2):
                        nc.tensor.matmul(
                            ps,
                            wc_sb[ct][:, dk * P:(dk + 1) * P],
                            den_sb[b][:, ct * NLOC:(ct + 1) * NLOC],
                            start=(ct == 0), stop=(ct == 1),
                        )
                    nc.scalar.activation(ot[:, dk, :], ps, AF.Identity,
                                         bias=bc_sb[dk])
                dst = out_h.ap()[b].rearrange("(dk p) n -> p dk n", dk=2)
                nc.sync.dma_start(out=dst, in_=ot)

    return nc


def _split_excess_waits(nc, mybir, cap=1):
    """The installed walrus rejects engine instructions carrying more than
    one semaphore wait; Tile emits up to 4.  Merge same-sem waits and hoist
    the excess onto single-wait EventSemaphore instructions just before, on
    the same engine queue."""
    n_ev = 0
    for fn in nc.m.functions:
        for blk in fn.blocks:
            insts = blk.instructions
            out = []
            changed = False
            for i in insts:
                si = getattr(i, "sync_info", None)
                waits = list(si.on_wait) if si is not None and si.on_wait else []
                if len(waits) > 1:
                    merged = {}
                    for w in waits:
                        k = w.id
                        if k not in merged or merged[k].wait_value < w.wait_value:
                            merged[k] = w
                    waits = list(merged.values())
                    while len(waits) > cap:
                        w = waits.pop(0)
                        ev = mybir.InstEventSemaphore(
                            name=f"{i.name}-wsplit{n_ev}", engine=i.engine)
                        ev.sync_info = mybir.SyncInfo(on_wait=[w], on_update=[])
                        try:
                            ev.debug = i.debug
                        except Exception:
                            pass
                        out.append(ev)
                        n_ev += 1
                    si.on_wait = waits
                    changed = True
                out.append(i)
            if changed:
                blk.instructions = out
    return n_ev


def _host_prep(x, w_theta, b_theta, w_phi, b_phi, w_conv, b_conv):
    import ml_dtypes

    bf16 = ml_dtypes.bfloat16
    x = np.asarray(x, dtype=np.float32)
    w_theta = np.asarray(w_theta, dtype=np.float32)
    b_theta = np.asarray(b_theta, dtype=np.float32)
    w_phi = np.asarray(w_phi, dtype=np.float32)
    b_phi = np.asarray(b_phi, dtype=np.float32)
    w_conv = np.asarray(w_conv, dtype=np.float32)
    b_conv = np.asarray(b_conv, dtype=np.float32)

    xr = x.reshape(B, C, HW)
    xtr = xr.transpose(0, 2, 1)
    # 1/sqrt(C) = 1/16: exact power-of-two scale folded into theta
    wthT = np.ascontiguousarray((w_theta * (1.0 / 16.0)).T).astype(bf16)
    wphT = np.ascontiguousarray(w_phi.T).astype(bf16)
    wcT = np.ascontiguousarray((np.eye(C, dtype=np.float32) + w_conv).T)
    bth = np.ascontiguousarray((b_theta * (1.0 / 16.0)).reshape(D, 1))
    bph = np.ascontiguousarray(b_phi.reshape(D, 1))
    bc = np.ascontiguousarray(b_conv.reshape(C, 1))

    xr16 = np.ascontiguousarray(xr).astype(bf16)
    xtr16 = np.ascontiguousarray(xtr).astype(bf16)

    in_maps = []
    for k in range(NCORES):
        rot = [(k + j) % NCHUNK for j in range(NCHUNK)]
        xn_k = np.concatenate(
            [xr16[:, :, r * MC:(r + 1) * MC] for r in rot], axis=2)
        xt_k = np.concatenate(
            [xtr16[:, r * MC:(r + 1) * MC, :] for r in rot], axis=1)
        in_maps.append({
            "xn": np.ascontiguousarray(xn_k),
            "xt": np.ascontiguousarray(xt_k),
            "wthT": wthT, "wphT": wphT, "wcT": wcT,
            "bth": bth, "bph": bph, "bc": bc,
        })
    return in_maps


def kernel(x, w_theta, b_theta, w_phi, b_phi, w_conv, b_conv):
    global _prog
    _ensure_path()
    from concourse.bass_utils import run_bass_kernel_spmd

    if _prog is None:
        _prog = _build()
        from concourse import mybir
        _split_excess_waits(_prog, mybir)

    in_maps = _host_prep(x, w_theta, b_theta, w_phi, b_phi, w_conv, b_conv)
    res = run_bass_kernel_spmd(
        _prog, in_maps, list(range(NCORES)), trace=TRACE
    )
    LAST["res"] = res

    outf = np.empty((B, C, HW), dtype=np.float32)
    for k in range(NCORES):
        outf[:, :, k * NLOC:(k + 1) * NLOC] = res.results[k]["out"]
    return outf.reshape(B, C, 64, 64)
